# revision 1
# baseline (speedup 1.0000x reference)
"""KV/KW cache scatter-update kernel for Trainium2 (8 NeuronCores, SPMD).

Semantics (matches the jax reference):
  pos = input_pos % S                      # (B, s)
  out_k[i]  = k_cache[batch_indexes[i]]  with out_k[i][:, pos[i,j], :] = k_val[i, :, j, :]
  out_v[i]  = likewise
  out_kw[i] = kw_cache[batch_indexes[i]] with out_kw[i][pos[i,j]] = kw_val[i, j]
  out_ks[i] = kw_sub_cache[...]          with out_ks[i][pos[i,j]] = kw_sub[i, j]

Sharding: batch-parallel. Core c owns batch slice batch_indexes[c]; all cache
traffic is local. The device kernel copies the cache slices input->output
(DRAM->DRAM DMA) and then overwrites the s=4 scattered seq positions with the
new values using dynamic-offset DMAs (positions read from a small int32
tensor on-device, so one SPMD program serves all cores).
"""

import os
import sys

import numpy as np

for _p in ("/opt/trn_rl_repo",):
    if os.path.isdir(_p) and _p not in sys.path:
        sys.path.insert(0, _p)

import concourse.bass as bass
import concourse.mybir as mybir
from concourse.bass_utils import run_bass_kernel_spmd

B, H, S, D = 8, 16, 4096, 128
s = 4
NCORES = 8
KW = 2 * H * H   # kw_cache trailing dims flattened: (2, H, H) -> 512
KS = 5 * 2 * H   # kw_sub_cache trailing dims flattened: (5, 2, H) -> 160


def build_nc(h=H, seq=S, d=D, sc=s, kw=KW, ks=KS):
    f32 = mybir.dt.float32
    i32 = mybir.dt.int32
    nc = bass.Bass(trn_type="TRN2")

    k_in = nc.dram_tensor("k_in", [h, seq, d], f32, kind="ExternalInput")
    v_in = nc.dram_tensor("v_in", [h, seq, d], f32, kind="ExternalInput")
    kw_in = nc.dram_tensor("kw_in", [seq, kw], f32, kind="ExternalInput")
    ks_in = nc.dram_tensor("ks_in", [seq, ks], f32, kind="ExternalInput")
    pos_in = nc.dram_tensor("pos_in", [1, sc], i32, kind="ExternalInput")
    k_val = nc.dram_tensor("k_val", [h, sc, d], f32, kind="ExternalInput")
    v_val = nc.dram_tensor("v_val", [h, sc, d], f32, kind="ExternalInput")
    kw_val = nc.dram_tensor("kw_val", [sc, kw], f32, kind="ExternalInput")
    ks_val = nc.dram_tensor("ks_val", [sc, ks], f32, kind="ExternalInput")

    k_out = nc.dram_tensor("k_out", [h, seq, d], f32, kind="ExternalOutput")
    v_out = nc.dram_tensor("v_out", [h, seq, d], f32, kind="ExternalOutput")
    kw_out = nc.dram_tensor("kw_out", [seq, kw], f32, kind="ExternalOutput")
    ks_out = nc.dram_tensor("ks_out", [seq, ks], f32, kind="ExternalOutput")

    with (
        nc.sbuf_tensor([1, sc], i32) as pos_sb,
        nc.Block() as block,
        nc.semaphore("pos_sem") as pos_sem,
        nc.semaphore("main_sem") as main_sem,
    ):

        @block.sync
        def _(sync):
            # positions into SBUF so the sequencer can read them
            sync.dma_start(pos_sb[:, :], pos_in[:, :]).then_inc(pos_sem, 16)

            # full cache copies, DRAM -> DRAM
            sync.dma_start(k_out[:, :, :], k_in[:, :, :]).then_inc(main_sem, 16)
            sync.dma_start(v_out[:, :, :], v_in[:, :, :]).then_inc(main_sem, 16)
            sync.dma_start(kw_out[:, :], kw_in[:, :]).then_inc(main_sem, 16)
            sync.dma_start(ks_out[:, :], ks_in[:, :]).then_inc(main_sem, 16)

            sync.wait_ge(pos_sem, 16)
            with (
                sync.register("p0") as r0,
                sync.register("p1") as r1,
                sync.register("p2") as r2,
                sync.register("p3") as r3,
            ):
                regs = [r0, r1, r2, r3][:sc]
                for j, r in enumerate(regs):
                    sync.reg_load(r, pos_sb[0:1, j : j + 1])

                # scatter writes must land after the full copies
                sync.wait_ge(main_sem, 64)
                n = 0
                for j, r in enumerate(regs):
                    p = sync.snap(r, min_val=0, max_val=seq - 1)
                    sync.dma_start(
                        k_out[:, bass.ds(p, 1), :], k_val[:, j : j + 1, :]
                    ).then_inc(main_sem, 16)
                    sync.dma_start(
                        v_out[:, bass.ds(p, 1), :], v_val[:, j : j + 1, :]
                    ).then_inc(main_sem, 16)
                    sync.dma_start(
                        kw_out[bass.ds(p, 1), :], kw_val[j : j + 1, :]
                    ).then_inc(main_sem, 16)
                    sync.dma_start(
                        ks_out[bass.ds(p, 1), :], ks_val[j : j + 1, :]
                    ).then_inc(main_sem, 16)
                    n += 4
                sync.wait_ge(main_sem, 64 + 16 * n)

    return nc


_NC = None


def _get_nc():
    global _NC
    if _NC is None:
        _NC = build_nc()
    return _NC


def _shard_inputs(
    k_cache, v_cache, kw_cache, kw_sub_cache, input_pos, batch_indexes,
    k_val, v_val, kw_val, kw_sub,
):
    pos = (input_pos.astype(np.int64) % S).astype(np.int32)
    bi = batch_indexes.astype(np.int64)
    in_maps = []
    for c in range(NCORES):
        b = int(bi[c])
        in_maps.append(
            {
                "k_in": np.ascontiguousarray(k_cache[b]),
                "v_in": np.ascontiguousarray(v_cache[b]),
                "kw_in": np.ascontiguousarray(kw_cache[b]).reshape(S, KW),
                "ks_in": np.ascontiguousarray(kw_sub_cache[b]).reshape(S, KS),
                "pos_in": np.ascontiguousarray(pos[c : c + 1]),
                "k_val": np.ascontiguousarray(k_val[c]),
                "v_val": np.ascontiguousarray(v_val[c]),
                "kw_val": np.ascontiguousarray(kw_val[c]).reshape(s, KW),
                "ks_val": np.ascontiguousarray(kw_sub[c]).reshape(s, KS),
            }
        )
    return in_maps


def run(inputs, trace=False, **kwargs):
    """Run on 8 cores. Returns ((k, v, kw, ks), BassKernelResults)."""
    inputs = {k: np.asarray(v) for k, v in inputs.items()}
    in_maps = _shard_inputs(**inputs)
    res = run_bass_kernel_spmd(
        _get_nc(), in_maps, core_ids=list(range(NCORES)), trace=trace, **kwargs
    )
    k = np.stack([r["k_out"] for r in res.results])
    v = np.stack([r["v_out"] for r in res.results])
    kw = np.stack([r["kw_out"] for r in res.results]).reshape(B, S, 2, H, H)
    ks = np.stack([r["ks_out"] for r in res.results]).reshape(B, S, 5, 2, H)
    return (k, v, kw, ks), res


def kernel(**inputs):
    outs, _ = run(inputs, trace=False)
    return outs


# revision 3
# speedup vs baseline: 1.2262x; 1.2262x over previous
"""KV/KW cache scatter-update kernel for Trainium2 (8 NeuronCores, SPMD).

Semantics (matches the jax reference):
  pos = input_pos % S                      # (B, s)
  out_k[i]  = k_cache[batch_indexes[i]]  with out_k[i][:, pos[i,j], :] = k_val[i, :, j, :]
  out_v[i]  = likewise
  out_kw[i] = kw_cache[batch_indexes[i]] with out_kw[i][pos[i,j]] = kw_val[i, j]
  out_ks[i] = kw_sub_cache[...]          with out_ks[i][pos[i,j]] = kw_sub[i, j]

Sharding: batch-parallel. Core c owns batch slice batch_indexes[c]; all cache
traffic is local. The device kernel copies the cache slices input->output
(DRAM->DRAM DMA) and then overwrites the s=4 scattered seq positions with the
new values using dynamic-offset DMAs (positions read from a small int32
tensor on-device, so one SPMD program serves all cores).
"""

import os
import sys

import numpy as np

for _p in ("/opt/trn_rl_repo",):
    if os.path.isdir(_p) and _p not in sys.path:
        sys.path.insert(0, _p)

import concourse.bass as bass
import concourse.mybir as mybir
from concourse.bass_utils import run_bass_kernel_spmd

B, H, S, D = 8, 16, 4096, 128
s = 4
NCORES = 8
KW = 2 * H * H   # kw_cache trailing dims flattened: (2, H, H) -> 512
KS = 5 * 2 * H   # kw_sub_cache trailing dims flattened: (5, 2, H) -> 160


def build_nc(h=H, seq=S, d=D, sc=s, kw=KW, ks=KS):
    """v2: cache copies split in half across both HWDGE rings (SP + ACT) so
    the 16 SDMA engines always have descriptors queued; dynamic-position
    scatters ride the empty SWDGE (gpsimd) ring, gated per-tensor so they
    overlap the remaining copies instead of forming a serial tail."""
    f32 = mybir.dt.float32
    i32 = mybir.dt.int32
    nc = bass.Bass(trn_type="TRN2")

    k_in = nc.dram_tensor("k_in", [h, seq, d], f32, kind="ExternalInput")
    v_in = nc.dram_tensor("v_in", [h, seq, d], f32, kind="ExternalInput")
    kw_in = nc.dram_tensor("kw_in", [seq, kw], f32, kind="ExternalInput")
    ks_in = nc.dram_tensor("ks_in", [seq, ks], f32, kind="ExternalInput")
    pos_in = nc.dram_tensor("pos_in", [1, sc], i32, kind="ExternalInput")
    k_val = nc.dram_tensor("k_val", [h, sc, d], f32, kind="ExternalInput")
    v_val = nc.dram_tensor("v_val", [h, sc, d], f32, kind="ExternalInput")
    kw_val = nc.dram_tensor("kw_val", [sc, kw], f32, kind="ExternalInput")
    ks_val = nc.dram_tensor("ks_val", [sc, ks], f32, kind="ExternalInput")

    k_out = nc.dram_tensor("k_out", [h, seq, d], f32, kind="ExternalOutput")
    v_out = nc.dram_tensor("v_out", [h, seq, d], f32, kind="ExternalOutput")
    kw_out = nc.dram_tensor("kw_out", [seq, kw], f32, kind="ExternalOutput")
    ks_out = nc.dram_tensor("ks_out", [seq, ks], f32, kind="ExternalOutput")

    h2 = h // 2
    q2 = seq // 2

    with (
        nc.sbuf_tensor([1, sc], i32) as pos_sb,
        nc.Block() as block,
        nc.semaphore("pos_sem") as pos_sem,
        nc.semaphore("sem_k") as sem_k,
        nc.semaphore("sem_v") as sem_v,
        nc.semaphore("sem_kw") as sem_kw,
        nc.semaphore("sem_ks") as sem_ks,
        nc.semaphore("scat_sem") as scat_sem,
    ):

        @block.sync
        def _(sync):
            # positions into SBUF for the scatter engine
            sync.dma_start(pos_sb[:, :], pos_in[:, :]).then_inc(pos_sem, 16)
            # first halves of every cache copy on the SP HWDGE ring
            sync.dma_start(k_out[:h2], k_in[:h2]).then_inc(sem_k, 16)
            sync.dma_start(v_out[:h2], v_in[:h2]).then_inc(sem_v, 16)
            sync.dma_start(kw_out[:q2], kw_in[:q2]).then_inc(sem_kw, 16)
            sync.dma_start(ks_out[:q2], ks_in[:q2]).then_inc(sem_ks, 16)

        @block.scalar
        def _(scalar):
            # second halves on the ACT HWDGE ring
            scalar.dma_start(k_out[h2:], k_in[h2:]).then_inc(sem_k, 16)
            scalar.dma_start(v_out[h2:], v_in[h2:]).then_inc(sem_v, 16)
            scalar.dma_start(kw_out[q2:], kw_in[q2:]).then_inc(sem_kw, 16)
            scalar.dma_start(ks_out[q2:], ks_in[q2:]).then_inc(sem_ks, 16)

        @block.gpsimd
        def _(gpsimd):
            gpsimd.wait_ge(pos_sem, 16)
            with (
                gpsimd.register("p0") as r0,
                gpsimd.register("p1") as r1,
                gpsimd.register("p2") as r2,
                gpsimd.register("p3") as r3,
            ):
                regs = [r0, r1, r2, r3][:sc]
                for j, r in enumerate(regs):
                    gpsimd.reg_load(r, pos_sb[0:1, j : j + 1])
                ps = [
                    gpsimd.snap(r, min_val=0, max_val=seq - 1) for r in regs
                ]

                # per-tensor: wait for both copy halves, then overwrite the
                # scattered rows; later tensors' copies still drain meanwhile
                gpsimd.wait_ge(sem_k, 32)
                for j, p in enumerate(ps):
                    gpsimd.dma_start(
                        k_out[:, bass.ds(p, 1), :], k_val[:, j : j + 1, :]
                    ).then_inc(scat_sem, 16)
                gpsimd.wait_ge(sem_v, 32)
                for j, p in enumerate(ps):
                    gpsimd.dma_start(
                        v_out[:, bass.ds(p, 1), :], v_val[:, j : j + 1, :]
                    ).then_inc(scat_sem, 16)
                gpsimd.wait_ge(sem_kw, 32)
                for j, p in enumerate(ps):
                    gpsimd.dma_start(
                        kw_out[bass.ds(p, 1), :], kw_val[j : j + 1, :]
                    ).then_inc(scat_sem, 16)
                gpsimd.wait_ge(sem_ks, 32)
                for j, p in enumerate(ps):
                    gpsimd.dma_start(
                        ks_out[bass.ds(p, 1), :], ks_val[j : j + 1, :]
                    ).then_inc(scat_sem, 16)

                gpsimd.wait_ge(scat_sem, 16 * 4 * sc)

    return nc


_NC = None


def _get_nc():
    global _NC
    if _NC is None:
        _NC = build_nc()
    return _NC


def _shard_inputs(
    k_cache, v_cache, kw_cache, kw_sub_cache, input_pos, batch_indexes,
    k_val, v_val, kw_val, kw_sub,
):
    pos = (input_pos.astype(np.int64) % S).astype(np.int32)
    bi = batch_indexes.astype(np.int64)
    in_maps = []
    for c in range(NCORES):
        b = int(bi[c])
        in_maps.append(
            {
                "k_in": np.ascontiguousarray(k_cache[b]),
                "v_in": np.ascontiguousarray(v_cache[b]),
                "kw_in": np.ascontiguousarray(kw_cache[b]).reshape(S, KW),
                "ks_in": np.ascontiguousarray(kw_sub_cache[b]).reshape(S, KS),
                "pos_in": np.ascontiguousarray(pos[c : c + 1]),
                "k_val": np.ascontiguousarray(k_val[c]),
                "v_val": np.ascontiguousarray(v_val[c]),
                "kw_val": np.ascontiguousarray(kw_val[c]).reshape(s, KW),
                "ks_val": np.ascontiguousarray(kw_sub[c]).reshape(s, KS),
            }
        )
    return in_maps


def run(inputs, trace=False, **kwargs):
    """Run on 8 cores. Returns ((k, v, kw, ks), BassKernelResults)."""
    inputs = {k: np.asarray(v) for k, v in inputs.items()}
    in_maps = _shard_inputs(**inputs)
    res = run_bass_kernel_spmd(
        _get_nc(), in_maps, core_ids=list(range(NCORES)), trace=trace, **kwargs
    )
    k = np.stack([r["k_out"] for r in res.results])
    v = np.stack([r["v_out"] for r in res.results])
    kw = np.stack([r["kw_out"] for r in res.results]).reshape(B, S, 2, H, H)
    ks = np.stack([r["ks_out"] for r in res.results]).reshape(B, S, 5, 2, H)
    return (k, v, kw, ks), res


def kernel(**inputs):
    outs, _ = run(inputs, trace=False)
    return outs
